# revision 18
# baseline (speedup 1.0000x reference)
"""Self-contained Trainium2 Bass kernel for the 2-layer GCN problem.

kernel(src, dst, vals, x, W1, W2) -> [80000, 40] float32 logits,
computed as  A @ (relu((A @ x) @ W1) @ W2)  on 8 NeuronCores.

v3 strategy:
- Nodes sharded round-robin across cores in 128-node slots (by src);
  W1/W2 replicated.
- Phase 1 (z1 = A@x): the per-edge gather of x[dst] is a pure input
  permutation, so the host pre-packs x rows into chunk-slot order and
  the device STREAMS them sequentially (no per-edge DMA descriptors).
  The selection matrices (one-hot(srel)*val per chunk) are likewise
  host-prebuilt and streamed (keeps DVE off the critical path).  A
  matmul per chunk scatter-adds slots into block rows in PSUM, then a
  dense chain per block: z2 = relu(z1 W1) W2 (64 padded cols).
- z2 shards AllGather into a superblock-major z2 table in 3 bank-sized
  pieces so phase-3 gathers of bank b start as soon as piece b lands
  (overlapping later phase-1/2 work).
- Phase 3 (out = A@z2): per-edge dma_gather of 256B z2 rows (4 SWDGE
  queues) + TRANSPOSED accumulation: matmul(lhsT=g[slot,feat],
  rhs=sv[slot,row]) -> accT[feat,row] per superblock in PSUM; bank
  partials summed into an SBUF accumulator.  Output leaves the device
  feature-major; the host transposes while assembling (free).
"""
import numpy as np
import ml_dtypes
import jax
from jax.sharding import Mesh, PartitionSpec, NamedSharding
from jax.experimental.shard_map import shard_map

import concourse.bass as bass
import concourse.bacc as bacc
import concourse.tile as tile
import concourse.mybir as mybir
from concourse import bass2jax
from concourse.bass2jax import _bass_exec_p, install_neuronx_cc_hook, partition_id_tensor
from concourse.masks import make_identity

NUM_NODES = 80000
NUM_EDGES = 1280000

NC, P, GRP = 8, 128, 1024
NBLK = 79            # ceil(80000 / 1024) blocks of 128 rows per core
SUPER = 8
NSB = 10             # ceil(NBLK / SUPER)
SLAB = 48

F_IN = 64
F_HID = 128
F_OUT = 40
FE = 128             # bf16 elements per 256B z2-table row

# z2 table bank structure (block-major): bank b covers blocks
# [BBLO[b], BBHI[b]); table row offset BBASE[b], per-core rows BROWS[b].
# The SMALL bank comes first so AG piece 0 (and with it the phase-3
# gather backbone) starts after only 15/79 of phase 1.
BBLO = [0, 15, 47]
BBHI = [15, 47, 79]
BBASE = [0, 15360, 48128]
BROWS = [1920, 4096, 4096]
NBANK = 3
TABROWS = 80896

bf16 = mybir.dt.bfloat16
f32 = mybir.dt.float32


def _cell_ranks(key, order_key2, nkey):
    """Rank of each edge within its integer cell `key` (cell-internal order
    by order_key2), plus per-cell counts."""
    ord_e = np.lexsort((order_key2, key))
    ks = key[ord_e]
    E = key.shape[0]
    first = np.r_[0, np.flatnonzero(np.diff(ks)) + 1]
    group_start = np.zeros(E, np.int64)
    group_start[first] = first
    group_start = np.maximum.accumulate(group_start)
    rank_sorted = np.arange(E) - group_start
    rank = np.empty(E, np.int64)
    rank[ord_e] = rank_sorted
    cnt = np.bincount(key, minlength=nkey)
    return rank, cnt


def build_layout(src, dst, vals, n_nodes=NUM_NODES):
    src = np.asarray(src, np.int64)
    dst = np.asarray(dst, np.int64)
    vals = np.asarray(vals, np.float32)

    n = np.arange(n_nodes)
    c_of = (n // P) % NC
    j_of = n // GRP
    s_of = n % P

    bank_arr = np.array(BBASE)
    jb = j_of
    bank_of = (jb >= BBLO[1]).astype(np.int64) + (jb >= BBLO[2]).astype(np.int64)
    bblo = np.array(BBLO)[bank_of]
    brows = np.array(BROWS)[bank_of]
    table_row = bank_arr[bank_of] + c_of * brows + (jb - bblo) * P + s_of

    ec, ej, es = c_of[src], j_of[src], s_of[src]
    ebank = bank_of[dst]
    lidx = (table_row[dst] - bank_arr[ebank]).astype(np.int64)

    # ---- phase 1 chunks: cell = (core, block) ----
    key1 = ec * NBLK + ej
    rank1, cnt1 = _cell_ranks(key1, dst, NC * NBLK)
    cnt1 = cnt1.reshape(NC, NBLK)
    K1 = np.maximum((-(-cnt1 // P)).max(axis=0), 1)        # [NBLK]
    CB1 = np.r_[0, np.cumsum(K1)]
    NCHUNK1 = int(CB1[-1])
    chunk_block1 = np.repeat(np.arange(NBLK), K1)
    first1 = CB1[:-1].copy()
    last1 = CB1[1:] - 1
    slot1 = (CB1[ej] + rank1 // P) * P + (rank1 % P)
    slabs1 = []
    for S in range(NSB):
        lo, hi = int(CB1[S * SUPER]), int(CB1[min((S + 1) * SUPER, NBLK)])
        o = lo
        while o < hi:
            take = min(SLAB, hi - o)
            slabs1.append((o, take, S))
            o += take

    # ---- phase 3 chunks: cell = (core, block, bank) ----
    key3 = (ec * NBLK + ej) * NBANK + ebank
    rank3, cnt3 = _cell_ranks(key3, lidx, NC * NBLK * NBANK)
    cnt3 = cnt3.reshape(NC, NBLK, NBANK)
    K3 = np.maximum((-(-cnt3 // P)).max(axis=0), 1)        # [NBLK, NBANK]
    CB3 = np.zeros((NBLK, NBANK), np.int64)
    chunk_block3 = []
    slabs3 = []
    pos = 0
    for b in range(NBANK):
        for S in range(NSB):
            js = range(S * SUPER, min((S + 1) * SUPER, NBLK))
            run_start = pos
            for j in js:
                CB3[j, b] = pos
                pos += int(K3[j, b])
                chunk_block3.extend([j] * int(K3[j, b]))
            o = run_start
            while o < pos:
                take = min(SLAB, pos - o)
                slabs3.append((o, take, b, S))
                o += take
    NCHUNK3 = int(pos)
    chunk_block3 = np.asarray(chunk_block3)
    first3 = CB3
    last3 = CB3 + K3 - 1
    slot3 = (CB3[ej, ebank] + rank3 // P) * P + (rank3 % P)

    # ---- per-core streams ----
    xg_dst = np.zeros((NC, NCHUNK1 * P), np.int64)
    v1 = np.zeros((NC, NCHUNK1 * P), np.float32)
    s1 = np.zeros((NC, NCHUNK1 * P), np.int64)
    # Pad slots must NOT all point at one row: same-address gathers
    # serialize in the DMA engines.  Spread them across the smallest
    # bank's row range (valid for every bank).
    pad_spread = (np.arange(NCHUNK3 * P, dtype=np.int64) * 97) % (NC * min(BROWS))
    gidx3 = np.tile(pad_spread.astype(np.int16), (NC, 1))
    v3 = np.zeros((NC, NCHUNK3 * P), np.float32)
    s3 = np.zeros((NC, NCHUNK3 * P), np.int64)
    for c in range(NC):
        m = ec == c
        sl = slot1[m]
        xg_dst[c, sl] = dst[m]
        v1[c, sl] = vals[m]
        s1[c, sl] = es[m]
        sl3 = slot3[m]
        gidx3[c, sl3] = lidx[m].astype(np.int16)
        v3[c, sl3] = vals[m]
        s3[c, sl3] = es[m]

    return dict(
        NCHUNK1=NCHUNK1, NCHUNK3=NCHUNK3,
        chunk_block1=chunk_block1, chunk_block3=chunk_block3,
        first1=first1, last1=last1, first3=first3, last3=last3,
        slabs1=slabs1, slabs3=slabs3,
        xg_dst=xg_dst, v1=v1, s1=s1,
        gidx3=gidx3, v3=v3, s3=s3,
        c_of=c_of, j_of=j_of, s_of=s_of,
    )


def build_nc(L):
    NCHUNK1, NCHUNK3 = L["NCHUNK1"], L["NCHUNK3"]
    slabs1, slabs3 = L["slabs1"], L["slabs3"]
    cb1, cb3 = L["chunk_block1"], L["chunk_block3"]
    first1, last1 = L["first1"], L["last1"]
    first3, last3 = L["first3"], L["last3"]

    nc = bacc.Bacc("TRN2", target_bir_lowering=False, debug=False, num_devices=NC,
                   num_swdge_queues=4)
    xg = nc.dram_tensor("xg", [NCHUNK1 * P, F_IN], bf16, kind="ExternalInput")
    svs1 = nc.dram_tensor("svs1", [NCHUNK1 * P, P], bf16, kind="ExternalInput")
    gidx3 = nc.dram_tensor("gidx3", [P, NCHUNK3 * 8], mybir.dt.int16, kind="ExternalInput")
    svs3 = nc.dram_tensor("svs3", [NCHUNK3 * P, P], bf16, kind="ExternalInput")
    w1 = nc.dram_tensor("w1", [F_IN, F_HID], bf16, kind="ExternalInput")
    w2 = nc.dram_tensor("w2", [F_HID, 64], bf16, kind="ExternalInput")
    out_ext = nc.dram_tensor("out", [F_IN, NBLK * P], f32, kind="ExternalOutput")

    with tile.TileContext(nc) as tc:
        with (
            tc.tile_pool(name="cons", bufs=1) as cons,
            tc.tile_pool(name="xgp", bufs=3) as xgp,
            tc.tile_pool(name="svt", bufs=3) as svtp,
            tc.tile_pool(name="g3p", bufs=6) as g3p,
            tc.tile_pool(name="dense", bufs=2) as dns,
            tc.tile_pool(name="osb", bufs=1) as osb,
            tc.tile_pool(name="psa", bufs=2, space="PSUM") as psa,
            tc.tile_pool(name="psb", bufs=1, space="PSUM") as psb,
            tc.tile_pool(name="psd", bufs=2, space="PSUM") as psd,
            tc.tile_pool(name="dram", bufs=1, space="DRAM") as dram,
        ):
            ident_t = cons.tile([P, P], bf16)
            make_identity(nc, ident_t[:])
            w1_t = cons.tile([F_IN, F_HID], bf16)
            w2_t = cons.tile([F_HID, 64], bf16)
            idx3_t = cons.tile([P, NCHUNK3 * 8], mybir.dt.int16)
            nc.sync.dma_start(out=w1_t[:], in_=w1[:, :])
            nc.sync.dma_start(out=w2_t[:], in_=w2[:, :])
            nc.sync.dma_start(out=idx3_t[:], in_=gidx3[:, :])

            z2loc = dram.tile([NBLK * P, FE], bf16)
            z2tab = dram.tile([TABROWS, FE], bf16)

            # ---- phase 1: z1 = A@x (streamed), dense chain, z2 shard ----
            def dense_block(j, acc_ap):
                z1_sb = dns.tile([P, F_IN], bf16, tag="z1")
                nc.vector.tensor_copy(out=z1_sb[:], in_=acc_ap)
                pt = psd.tile([F_IN, P], bf16, tag="pt")
                nc.tensor.transpose(out=pt[:], in_=z1_sb[:], identity=ident_t[:])
                z1t = dns.tile([F_IN, P], bf16, tag="z1t")
                nc.vector.tensor_copy(out=z1t[:], in_=pt[:])
                ph = psd.tile([F_HID, P], f32, tag="pd")
                nc.tensor.matmul(out=ph[:], lhsT=w1_t[:], rhs=z1t[:],
                                 start=True, stop=True)
                ht = dns.tile([F_HID, P], bf16, tag="ht")
                nc.vector.tensor_scalar_max(out=ht[:], in0=ph[:], scalar1=0.0)
                pz = psd.tile([P, 64], f32, tag="pd")
                nc.tensor.matmul(out=pz[:], lhsT=ht[:], rhs=w2_t[:],
                                 start=True, stop=True)
                z2_sb = dns.tile([P, 64], bf16, tag="z2")
                nc.vector.tensor_copy(out=z2_sb[:], in_=pz[:])
                nc.sync.dma_start(out=z2loc[j * P:(j + 1) * P, 0:64], in_=z2_sb[:])

            for S in range(NSB):
                jlo = S * SUPER
                jhi = min((S + 1) * SUPER, NBLK)
                acc = psa.tile([P, 64 * SUPER], f32, tag="acc")
                for (c0, Ln, SS) in slabs1:
                    if SS != S:
                        continue
                    g1 = xgp.tile([P, SLAB, F_IN], bf16, tag="xg")
                    nc.sync.dma_start(
                        out=g1[:, 0:Ln, :],
                        in_=xg[c0 * P:(c0 + Ln) * P, :].rearrange(
                            "(p a) f -> p a f", p=P),
                    )
                    svt = svtp.tile([P, SLAB, P], bf16, tag="svt")
                    sv1_eng = nc.sync if (c0 // SLAB) % 3 == 2 else nc.scalar
                    sv1_eng.dma_start(
                        out=svt[:, 0:Ln, :],
                        in_=svs1[c0 * P:(c0 + Ln) * P, :].rearrange(
                            "(p a) f -> p a f", p=P),
                    )
                    for t in range(Ln):
                        ch = c0 + t
                        j = int(cb1[ch])
                        jj = j - jlo
                        nc.tensor.matmul(
                            out=acc[:, 64 * jj:64 * (jj + 1)],
                            lhsT=svt[:, t, :],
                            rhs=g1[:, t, :],
                            start=(ch == first1[j]),
                            stop=(ch == last1[j]),
                            skip_group_check=True,
                        )
                for j in range(jlo, jhi):
                    dense_block(j, acc[:, 64 * (j - jlo):64 * (j - jlo + 1)])

            # ---- phases 2+3 interleaved per bank ----
            out_sb = osb.tile([F_IN, NBLK * P], f32)
            for b in range(NBANK):
                nc.gpsimd.collective_compute(
                    "AllGather",
                    mybir.AluOpType.bypass,
                    replica_groups=[list(range(NC))],
                    ins=[z2loc[BBLO[b] * P:BBHI[b] * P, :].opt()],
                    outs=[z2tab[BBASE[b]:BBASE[b] + NC * BROWS[b], :].opt()],
                )
                for S in range(NSB):
                    jlo = S * SUPER
                    jhi = min((S + 1) * SUPER, NBLK)
                    accT = psb.tile([F_IN, P * SUPER], f32, tag="accT")
                    for si, (c0, Ln, bb, SS) in enumerate(slabs3):
                        if bb != b or SS != S:
                            continue
                        g3 = g3p.tile([P, SLAB, FE], bf16, tag="g3")
                        nc.gpsimd.dma_gather(
                            g3[:, 0:Ln, :],
                            z2tab[BBASE[b]:BBASE[b] + NC * BROWS[b], :],
                            idx3_t[:, c0 * 8:(c0 + Ln) * 8],
                            Ln * P, Ln * P, FE,
                            single_packet=False,
                            queue_num=(si % 4),
                        )
                        svt = svtp.tile([P, SLAB, P], bf16, tag="svt")
                        sv_eng = nc.scalar if (si % 2 == 0) else nc.sync
                        sv_eng.dma_start(
                            out=svt[:, 0:Ln, :],
                            in_=svs3[c0 * P:(c0 + Ln) * P, :].rearrange(
                                "(p a) f -> p a f", p=P),
                        )
                        for t in range(Ln):
                            ch = c0 + t
                            j = int(cb3[ch])
                            jj = j - jlo
                            nc.tensor.matmul(
                                out=accT[:, P * jj:P * (jj + 1)],
                                lhsT=g3[:, t, 0:F_IN],
                                rhs=svt[:, t, :],
                                start=(ch == first3[j, b]),
                                stop=(ch == last3[j, b]),
                                skip_group_check=True,
                            )
                    ncols = P * (jhi - jlo)
                    dst_ap = out_sb[:, P * jlo:P * jhi]
                    if b == 0:
                        nc.vector.tensor_copy(out=dst_ap, in_=accT[:, 0:ncols])
                    elif b == NBANK - 1:
                        nc.vector.tensor_add(out=dst_ap, in0=dst_ap,
                                             in1=accT[:, 0:ncols])
                        nc.sync.dma_start(out=out_ext[:, P * jlo:P * jhi],
                                          in_=dst_ap)
                    else:
                        nc.vector.tensor_add(out=dst_ap, in0=dst_ap,
                                             in1=accT[:, 0:ncols])

    nc.compile()
    return nc


def _slabify(rows, slabs, fdim):
    """Reorder stream rows so each slab is partition-major contiguous:
    dram row slab_base*P + s*Ln + a  <-  slot (c0+a)*P + s."""
    out = np.empty_like(rows)
    for sl in slabs:
        c0, Ln = sl[0], sl[1]
        blk = rows[c0 * P:(c0 + Ln) * P].reshape(Ln, P, fdim)
        out[c0 * P:(c0 + Ln) * P] = blk.transpose(1, 0, 2).reshape(Ln * P, fdim)
    return out


def _sv_rows(s, v, nslots):
    rows = np.zeros((nslots, P), np.float32)
    rows[np.arange(nslots), s] = v
    return rows.astype(ml_dtypes.bfloat16)


def pack_inputs(L, x, W1, W2):
    NCHUNK1, NCHUNK3 = L["NCHUNK1"], L["NCHUNK3"]
    x = np.asarray(x, np.float32)
    w1b = np.asarray(W1, np.float32).astype(ml_dtypes.bfloat16)
    w2b = np.zeros((F_HID, 64), ml_dtypes.bfloat16)
    w2b[:, 0:F_OUT] = np.asarray(W2, np.float32).astype(ml_dtypes.bfloat16)

    xbf = x.astype(ml_dtypes.bfloat16)
    in_maps = []
    for c in range(NC):
        xg_rows = _slabify(xbf[L["xg_dst"][c]], L["slabs1"], F_IN)
        sv1_rows = _slabify(_sv_rows(L["s1"][c], L["v1"][c], NCHUNK1 * P),
                            L["slabs1"], P)
        sv3_rows = _slabify(_sv_rows(L["s3"][c], L["v3"][c], NCHUNK3 * P),
                            L["slabs3"], P)
        n3 = NCHUNK3 * P
        wrap = L["gidx3"][c].reshape(n3 // 16, 16).T      # [16, n/16]
        idx_tile = np.tile(wrap, (8, 1)).copy()
        in_maps.append({
            "xg": xg_rows, "svs1": sv1_rows,
            "gidx3": idx_tile, "svs3": sv3_rows,
            "w1": w1b, "w2": w2b,
        })
    return in_maps


def unpack_output(L, results):
    """results: per-core dicts with 'out' [64, NBLK*P] (feature-major)."""
    big = np.stack([r["out"] for r in results])           # [NC, 64, NBLK*P]
    col = L["j_of"] * P + L["s_of"]
    out = big[L["c_of"], :, col]                          # [NUM_NODES, 64]
    return np.ascontiguousarray(out[:, 0:F_OUT])


def make_runner(nc, n_cores=8, donate=False):
    install_neuronx_cc_hook()
    partition_name = nc.partition_id_tensor.name if nc.partition_id_tensor else None

    in_names, out_names, out_avals, zero_outs = [], [], [], []
    for alloc in nc.m.functions[0].allocations:
        if not isinstance(alloc, mybir.MemoryLocationSet):
            continue
        name = alloc.memorylocations[0].name
        if alloc.kind == "ExternalInput":
            if name != partition_name:
                in_names.append(name)
        elif alloc.kind == "ExternalOutput":
            out_names.append(name)
            shape = tuple(alloc.tensor_shape)
            dtype = mybir.dt.np(alloc.dtype)
            out_avals.append(jax.core.ShapedArray(shape, dtype))
            zero_outs.append(np.zeros(shape, dtype))
    n_params = len(in_names)
    n_outs = len(out_avals)
    all_in_names = list(in_names) + list(out_names)
    if partition_name is not None:
        all_in_names.append(partition_name)

    def _body(*args):
        operands = list(args)
        if partition_name is not None:
            operands.append(partition_id_tensor())
        outs = _bass_exec_p.bind(
            *operands,
            out_avals=tuple(out_avals),
            in_names=tuple(all_in_names),
            out_names=tuple(out_names),
            lowering_input_output_aliases=(),
            sim_require_finite=True,
            sim_require_nnan=True,
            nc=nc,
        )
        return tuple(outs)

    devices = jax.devices()[:n_cores]
    mesh = Mesh(np.asarray(devices), ("core",))
    in_specs = (PartitionSpec("core"),) * (n_params + n_outs)
    out_specs = (PartitionSpec("core"),) * n_outs
    jit_kwargs = {"keep_unused": True}
    if donate:
        jit_kwargs["donate_argnums"] = tuple(range(n_params, n_params + n_outs))
    fn = jax.jit(
        shard_map(_body, mesh=mesh, in_specs=in_specs, out_specs=out_specs,
                  check_rep=False),
        **jit_kwargs,
    )
    sharding = NamedSharding(mesh, PartitionSpec("core"))

    class Runner:
        def __init__(self):
            self.fn = fn
            self.in_names = in_names
            self.out_names = out_names
            self.n_cores = n_cores
            self.sharding = sharding
            self.zero_outs = zero_outs

        def put_inputs(self, in_maps):
            args = []
            for name in in_names:
                cat = np.concatenate([np.asarray(m[name]) for m in in_maps], axis=0)
                args.append(jax.device_put(cat, sharding))
            for z in zero_outs:
                cat = np.concatenate([z] * n_cores, axis=0)
                args.append(jax.device_put(cat, sharding))
            return args

        def __call__(self, args):
            return self.fn(*args)

        def run(self, in_maps):
            args = self.put_inputs(in_maps)
            outs = self.fn(*args)
            jax.block_until_ready(outs)
            res = []
            for c in range(n_cores):
                d = {}
                for i, name in enumerate(out_names):
                    arr = np.asarray(outs[i])
                    per = arr.shape[0] // n_cores
                    d[name] = arr[c * per:(c + 1) * per]
                res.append(d)
            return res

    return Runner()


_CACHE = {}


def kernel(src, dst, vals, x, W1, W2):
    src = np.asarray(src)
    dst = np.asarray(dst)
    vals = np.asarray(vals, dtype=np.float32)
    x = np.asarray(x, dtype=np.float32)
    W1 = np.asarray(W1, dtype=np.float32)
    W2 = np.asarray(W2, dtype=np.float32)

    L = build_layout(src.astype(np.int64), dst.astype(np.int64), vals, NUM_NODES)
    key = (L["NCHUNK1"], L["NCHUNK3"], tuple(L["slabs1"]),
           tuple(L["slabs3"]), tuple(L["first1"]),
           tuple(map(tuple, L["first3"])))
    if key not in _CACHE:
        nc = build_nc(L)
        _CACHE[key] = make_runner(nc)
        _CACHE["r"] = _CACHE[key]
    r = _CACHE[key]
    in_maps = pack_inputs(L, x, W1, W2)
    results = r.run(in_maps)
    return unpack_output(L, results).astype(np.float32)


# revision 19
# speedup vs baseline: 1.0800x; 1.0800x over previous
"""Self-contained Trainium2 Bass kernel for the 2-layer GCN problem.

kernel(src, dst, vals, x, W1, W2) -> [80000, 40] float32 logits,
computed as  A @ (relu((A @ x) @ W1) @ W2)  on 8 NeuronCores.

v3 strategy:
- Nodes sharded round-robin across cores in 128-node slots (by src);
  W1/W2 replicated.
- Phase 1 (z1 = A@x): the per-edge gather of x[dst] is a pure input
  permutation, so the host pre-packs x rows into chunk-slot order and
  the device STREAMS them sequentially (no per-edge DMA descriptors).
  The selection matrices (one-hot(srel)*val per chunk) are likewise
  host-prebuilt and streamed (keeps DVE off the critical path).  A
  matmul per chunk scatter-adds slots into block rows in PSUM, then a
  dense chain per block: z2 = relu(z1 W1) W2 (64 padded cols).
- z2 shards AllGather into a superblock-major z2 table in 3 bank-sized
  pieces so phase-3 gathers of bank b start as soon as piece b lands
  (overlapping later phase-1/2 work).
- Phase 3 (out = A@z2): per-edge dma_gather of 256B z2 rows (4 SWDGE
  queues) + TRANSPOSED accumulation: matmul(lhsT=g[slot,feat],
  rhs=sv[slot,row]) -> accT[feat,row] per superblock in PSUM; bank
  partials summed into an SBUF accumulator.  Output leaves the device
  feature-major; the host transposes while assembling (free).
"""
import numpy as np
import ml_dtypes
import jax
from jax.sharding import Mesh, PartitionSpec, NamedSharding
from jax.experimental.shard_map import shard_map

import concourse.bass as bass
import concourse.bacc as bacc
import concourse.tile as tile
import concourse.mybir as mybir
from concourse import bass2jax
from concourse.bass2jax import _bass_exec_p, install_neuronx_cc_hook, partition_id_tensor
from concourse.masks import make_identity

NUM_NODES = 80000
NUM_EDGES = 1280000

NC, P, GRP = 8, 128, 1024
NBLK = 79            # ceil(80000 / 1024) blocks of 128 rows per core
SUPER = 8
NSB = 10             # ceil(NBLK / SUPER)
SLAB = 48

F_IN = 64
F_HID = 128
F_OUT = 40
FE = 128             # bf16 elements per 256B z2-table row

# z2 table bank structure (block-major): bank b covers blocks
# [BBLO[b], BBHI[b]); table row offset BBASE[b], per-core rows BROWS[b].
# The SMALL bank comes first so AG piece 0 (and with it the phase-3
# gather backbone) starts after only 15/79 of phase 1.
BBLO = [0, 15, 47]
BBHI = [15, 47, 79]
BBASE = [0, 15360, 48128]
BROWS = [1920, 4096, 4096]
NBANK = 3
TABROWS = 80896

bf16 = mybir.dt.bfloat16
f32 = mybir.dt.float32


def _cell_ranks(key, order_key2, nkey):
    """Rank of each edge within its integer cell `key` (cell-internal order
    by order_key2), plus per-cell counts."""
    ord_e = np.lexsort((order_key2, key))
    ks = key[ord_e]
    E = key.shape[0]
    first = np.r_[0, np.flatnonzero(np.diff(ks)) + 1]
    group_start = np.zeros(E, np.int64)
    group_start[first] = first
    group_start = np.maximum.accumulate(group_start)
    rank_sorted = np.arange(E) - group_start
    rank = np.empty(E, np.int64)
    rank[ord_e] = rank_sorted
    cnt = np.bincount(key, minlength=nkey)
    return rank, cnt


def build_layout(src, dst, vals, n_nodes=NUM_NODES):
    src = np.asarray(src, np.int64)
    dst = np.asarray(dst, np.int64)
    vals = np.asarray(vals, np.float32)

    n = np.arange(n_nodes)
    c_of = (n // P) % NC
    j_of = n // GRP
    s_of = n % P

    bank_arr = np.array(BBASE)
    jb = j_of
    bank_of = (jb >= BBLO[1]).astype(np.int64) + (jb >= BBLO[2]).astype(np.int64)
    bblo = np.array(BBLO)[bank_of]
    brows = np.array(BROWS)[bank_of]
    table_row = bank_arr[bank_of] + c_of * brows + (jb - bblo) * P + s_of

    ec, ej, es = c_of[src], j_of[src], s_of[src]
    ebank = bank_of[dst]
    lidx = (table_row[dst] - bank_arr[ebank]).astype(np.int64)

    # ---- phase 1 chunks: cell = (core, block) ----
    key1 = ec * NBLK + ej
    rank1, cnt1 = _cell_ranks(key1, dst, NC * NBLK)
    cnt1 = cnt1.reshape(NC, NBLK)
    K1 = np.maximum((-(-cnt1 // P)).max(axis=0), 1)        # [NBLK]
    CB1 = np.r_[0, np.cumsum(K1)]
    NCHUNK1 = int(CB1[-1])
    chunk_block1 = np.repeat(np.arange(NBLK), K1)
    first1 = CB1[:-1].copy()
    last1 = CB1[1:] - 1
    slot1 = (CB1[ej] + rank1 // P) * P + (rank1 % P)
    slabs1 = []
    for S in range(NSB):
        lo, hi = int(CB1[S * SUPER]), int(CB1[min((S + 1) * SUPER, NBLK)])
        o = lo
        while o < hi:
            take = min(SLAB, hi - o)
            slabs1.append((o, take, S))
            o += take

    # ---- phase 3 chunks: cell = (core, block, bank) ----
    key3 = (ec * NBLK + ej) * NBANK + ebank
    rank3, cnt3 = _cell_ranks(key3, lidx, NC * NBLK * NBANK)
    cnt3 = cnt3.reshape(NC, NBLK, NBANK)
    K3 = np.maximum((-(-cnt3 // P)).max(axis=0), 1)        # [NBLK, NBANK]
    CB3 = np.zeros((NBLK, NBANK), np.int64)
    chunk_block3 = []
    slabs3 = []
    pos = 0
    for b in range(NBANK):
        for S in range(NSB):
            js = range(S * SUPER, min((S + 1) * SUPER, NBLK))
            run_start = pos
            for j in js:
                CB3[j, b] = pos
                pos += int(K3[j, b])
                chunk_block3.extend([j] * int(K3[j, b]))
            o = run_start
            while o < pos:
                take = min(SLAB, pos - o)
                slabs3.append((o, take, b, S))
                o += take
    NCHUNK3 = int(pos)
    chunk_block3 = np.asarray(chunk_block3)
    first3 = CB3
    last3 = CB3 + K3 - 1
    slot3 = (CB3[ej, ebank] + rank3 // P) * P + (rank3 % P)

    # ---- per-core streams ----
    xg_dst = np.zeros((NC, NCHUNK1 * P), np.int64)
    v1 = np.zeros((NC, NCHUNK1 * P), np.float32)
    s1 = np.zeros((NC, NCHUNK1 * P), np.int64)
    # Pad slots must NOT all point at one row: same-address gathers
    # serialize in the DMA engines.  Spread them across the smallest
    # bank's row range (valid for every bank).
    pad_spread = (np.arange(NCHUNK3 * P, dtype=np.int64) * 97) % (NC * min(BROWS))
    gidx3 = np.tile(pad_spread.astype(np.int16), (NC, 1))
    v3 = np.zeros((NC, NCHUNK3 * P), np.float32)
    s3 = np.zeros((NC, NCHUNK3 * P), np.int64)
    for c in range(NC):
        m = ec == c
        sl = slot1[m]
        xg_dst[c, sl] = dst[m]
        v1[c, sl] = vals[m]
        s1[c, sl] = es[m]
        sl3 = slot3[m]
        gidx3[c, sl3] = lidx[m].astype(np.int16)
        v3[c, sl3] = vals[m]
        s3[c, sl3] = es[m]

    return dict(
        NCHUNK1=NCHUNK1, NCHUNK3=NCHUNK3,
        chunk_block1=chunk_block1, chunk_block3=chunk_block3,
        first1=first1, last1=last1, first3=first3, last3=last3,
        slabs1=slabs1, slabs3=slabs3,
        xg_dst=xg_dst, v1=v1, s1=s1,
        gidx3=gidx3, v3=v3, s3=s3,
        c_of=c_of, j_of=j_of, s_of=s_of,
    )


def build_nc(L):
    NCHUNK1, NCHUNK3 = L["NCHUNK1"], L["NCHUNK3"]
    slabs1, slabs3 = L["slabs1"], L["slabs3"]
    cb1, cb3 = L["chunk_block1"], L["chunk_block3"]
    first1, last1 = L["first1"], L["last1"]
    first3, last3 = L["first3"], L["last3"]

    nc = bacc.Bacc("TRN2", target_bir_lowering=False, debug=False, num_devices=NC,
                   num_swdge_queues=4)
    xg = nc.dram_tensor("xg", [NCHUNK1 * P, F_IN], bf16, kind="ExternalInput")
    svs1 = nc.dram_tensor("svs1", [NCHUNK1 * P, P], bf16, kind="ExternalInput")
    gidx3 = nc.dram_tensor("gidx3", [P, NCHUNK3 * 8], mybir.dt.int16, kind="ExternalInput")
    svs3 = nc.dram_tensor("svs3", [NCHUNK3 * P, P], bf16, kind="ExternalInput")
    w1 = nc.dram_tensor("w1", [F_IN, F_HID], bf16, kind="ExternalInput")
    w2 = nc.dram_tensor("w2", [F_HID, 64], bf16, kind="ExternalInput")
    out_ext = nc.dram_tensor("out", [F_IN, NBLK * P], f32, kind="ExternalOutput")

    with tile.TileContext(nc) as tc:
        with (
            tc.tile_pool(name="cons", bufs=1) as cons,
            tc.tile_pool(name="xgp", bufs=3) as xgp,
            tc.tile_pool(name="svt", bufs=4) as svtp,
            tc.tile_pool(name="g3p", bufs=6) as g3p,
            tc.tile_pool(name="dense", bufs=2) as dns,
            tc.tile_pool(name="osb", bufs=1) as osb,
            tc.tile_pool(name="psa", bufs=2, space="PSUM") as psa,
            tc.tile_pool(name="psb", bufs=1, space="PSUM") as psb,
            tc.tile_pool(name="psd", bufs=2, space="PSUM") as psd,
            tc.tile_pool(name="dram", bufs=1, space="DRAM") as dram,
        ):
            ident_t = cons.tile([P, P], bf16)
            make_identity(nc, ident_t[:])
            w1_t = cons.tile([F_IN, F_HID], bf16)
            w2_t = cons.tile([F_HID, 64], bf16)
            idx3_t = cons.tile([P, NCHUNK3 * 8], mybir.dt.int16)
            nc.sync.dma_start(out=w1_t[:], in_=w1[:, :])
            nc.sync.dma_start(out=w2_t[:], in_=w2[:, :])
            nc.sync.dma_start(out=idx3_t[:], in_=gidx3[:, :])

            z2loc = dram.tile([NBLK * P, FE], bf16)
            z2tab = dram.tile([TABROWS, FE], bf16)

            # ---- phase 1: z1 = A@x (streamed), dense chain, z2 shard ----
            def dense_block(j, acc_ap):
                z1_sb = dns.tile([P, F_IN], bf16, tag="z1")
                nc.vector.tensor_copy(out=z1_sb[:], in_=acc_ap)
                pt = psd.tile([F_IN, P], bf16, tag="pt")
                nc.tensor.transpose(out=pt[:], in_=z1_sb[:], identity=ident_t[:])
                z1t = dns.tile([F_IN, P], bf16, tag="z1t")
                nc.vector.tensor_copy(out=z1t[:], in_=pt[:])
                ph = psd.tile([F_HID, P], f32, tag="pd")
                nc.tensor.matmul(out=ph[:], lhsT=w1_t[:], rhs=z1t[:],
                                 start=True, stop=True)
                ht = dns.tile([F_HID, P], bf16, tag="ht")
                nc.vector.tensor_scalar_max(out=ht[:], in0=ph[:], scalar1=0.0)
                pz = psd.tile([P, 64], f32, tag="pd")
                nc.tensor.matmul(out=pz[:], lhsT=ht[:], rhs=w2_t[:],
                                 start=True, stop=True)
                z2_sb = dns.tile([P, 64], bf16, tag="z2")
                nc.vector.tensor_copy(out=z2_sb[:], in_=pz[:])
                nc.sync.dma_start(out=z2loc[j * P:(j + 1) * P, 0:64], in_=z2_sb[:])

            for S in range(NSB):
                jlo = S * SUPER
                jhi = min((S + 1) * SUPER, NBLK)
                acc = psa.tile([P, 64 * SUPER], f32, tag="acc")
                for (c0, Ln, SS) in slabs1:
                    if SS != S:
                        continue
                    g1 = xgp.tile([P, SLAB, F_IN], bf16, tag="xg")
                    nc.sync.dma_start(
                        out=g1[:, 0:Ln, :],
                        in_=xg[c0 * P:(c0 + Ln) * P, :].rearrange(
                            "(p a) f -> p a f", p=P),
                    )
                    svt = svtp.tile([P, SLAB, P], bf16, tag="svt")
                    sv1_eng = nc.sync if (c0 // SLAB) % 3 == 2 else nc.scalar
                    sv1_eng.dma_start(
                        out=svt[:, 0:Ln, :],
                        in_=svs1[c0 * P:(c0 + Ln) * P, :].rearrange(
                            "(p a) f -> p a f", p=P),
                    )
                    for t in range(Ln):
                        ch = c0 + t
                        j = int(cb1[ch])
                        jj = j - jlo
                        nc.tensor.matmul(
                            out=acc[:, 64 * jj:64 * (jj + 1)],
                            lhsT=svt[:, t, :],
                            rhs=g1[:, t, :],
                            start=(ch == first1[j]),
                            stop=(ch == last1[j]),
                            skip_group_check=True,
                        )
                for j in range(jlo, jhi):
                    dense_block(j, acc[:, 64 * (j - jlo):64 * (j - jlo + 1)])

            # ---- phases 2+3 interleaved per bank ----
            out_sb = osb.tile([F_IN, NBLK * P], f32)
            for b in range(NBANK):
                nc.gpsimd.collective_compute(
                    "AllGather",
                    mybir.AluOpType.bypass,
                    replica_groups=[list(range(NC))],
                    ins=[z2loc[BBLO[b] * P:BBHI[b] * P, :].opt()],
                    outs=[z2tab[BBASE[b]:BBASE[b] + NC * BROWS[b], :].opt()],
                )
                for S in range(NSB):
                    jlo = S * SUPER
                    jhi = min((S + 1) * SUPER, NBLK)
                    accT = psb.tile([F_IN, P * SUPER], f32, tag="accT")
                    for si, (c0, Ln, bb, SS) in enumerate(slabs3):
                        if bb != b or SS != S:
                            continue
                        g3 = g3p.tile([P, SLAB, FE], bf16, tag="g3")
                        nc.gpsimd.dma_gather(
                            g3[:, 0:Ln, :],
                            z2tab[BBASE[b]:BBASE[b] + NC * BROWS[b], :],
                            idx3_t[:, c0 * 8:(c0 + Ln) * 8],
                            Ln * P, Ln * P, FE,
                            single_packet=False,
                            queue_num=(si % 4),
                        )
                        svt = svtp.tile([P, SLAB, P], bf16, tag="svt")
                        sv_eng = nc.scalar if (si % 2 == 0) else nc.sync
                        sv_eng.dma_start(
                            out=svt[:, 0:Ln, :],
                            in_=svs3[c0 * P:(c0 + Ln) * P, :].rearrange(
                                "(p a) f -> p a f", p=P),
                        )
                        for t in range(Ln):
                            ch = c0 + t
                            j = int(cb3[ch])
                            jj = j - jlo
                            nc.tensor.matmul(
                                out=accT[:, P * jj:P * (jj + 1)],
                                lhsT=g3[:, t, 0:F_IN],
                                rhs=svt[:, t, :],
                                start=(ch == first3[j, b]),
                                stop=(ch == last3[j, b]),
                                skip_group_check=True,
                            )
                    ncols = P * (jhi - jlo)
                    dst_ap = out_sb[:, P * jlo:P * jhi]
                    if b == 0:
                        nc.vector.tensor_copy(out=dst_ap, in_=accT[:, 0:ncols])
                    elif b == NBANK - 1:
                        nc.vector.tensor_add(out=dst_ap, in0=dst_ap,
                                             in1=accT[:, 0:ncols])
                        nc.sync.dma_start(out=out_ext[:, P * jlo:P * jhi],
                                          in_=dst_ap)
                    else:
                        nc.vector.tensor_add(out=dst_ap, in0=dst_ap,
                                             in1=accT[:, 0:ncols])

    nc.compile()
    return nc


def _slabify(rows, slabs, fdim):
    """Reorder stream rows so each slab is partition-major contiguous:
    dram row slab_base*P + s*Ln + a  <-  slot (c0+a)*P + s."""
    out = np.empty_like(rows)
    for sl in slabs:
        c0, Ln = sl[0], sl[1]
        blk = rows[c0 * P:(c0 + Ln) * P].reshape(Ln, P, fdim)
        out[c0 * P:(c0 + Ln) * P] = blk.transpose(1, 0, 2).reshape(Ln * P, fdim)
    return out


def _sv_rows(s, v, nslots):
    rows = np.zeros((nslots, P), np.float32)
    rows[np.arange(nslots), s] = v
    return rows.astype(ml_dtypes.bfloat16)


def pack_inputs(L, x, W1, W2):
    NCHUNK1, NCHUNK3 = L["NCHUNK1"], L["NCHUNK3"]
    x = np.asarray(x, np.float32)
    w1b = np.asarray(W1, np.float32).astype(ml_dtypes.bfloat16)
    w2b = np.zeros((F_HID, 64), ml_dtypes.bfloat16)
    w2b[:, 0:F_OUT] = np.asarray(W2, np.float32).astype(ml_dtypes.bfloat16)

    xbf = x.astype(ml_dtypes.bfloat16)
    in_maps = []
    for c in range(NC):
        xg_rows = _slabify(xbf[L["xg_dst"][c]], L["slabs1"], F_IN)
        sv1_rows = _slabify(_sv_rows(L["s1"][c], L["v1"][c], NCHUNK1 * P),
                            L["slabs1"], P)
        sv3_rows = _slabify(_sv_rows(L["s3"][c], L["v3"][c], NCHUNK3 * P),
                            L["slabs3"], P)
        n3 = NCHUNK3 * P
        wrap = L["gidx3"][c].reshape(n3 // 16, 16).T      # [16, n/16]
        idx_tile = np.tile(wrap, (8, 1)).copy()
        in_maps.append({
            "xg": xg_rows, "svs1": sv1_rows,
            "gidx3": idx_tile, "svs3": sv3_rows,
            "w1": w1b, "w2": w2b,
        })
    return in_maps


def unpack_output(L, results):
    """results: per-core dicts with 'out' [64, NBLK*P] (feature-major)."""
    big = np.stack([r["out"] for r in results])           # [NC, 64, NBLK*P]
    col = L["j_of"] * P + L["s_of"]
    out = big[L["c_of"], :, col]                          # [NUM_NODES, 64]
    return np.ascontiguousarray(out[:, 0:F_OUT])


def make_runner(nc, n_cores=8, donate=False):
    install_neuronx_cc_hook()
    partition_name = nc.partition_id_tensor.name if nc.partition_id_tensor else None

    in_names, out_names, out_avals, zero_outs = [], [], [], []
    for alloc in nc.m.functions[0].allocations:
        if not isinstance(alloc, mybir.MemoryLocationSet):
            continue
        name = alloc.memorylocations[0].name
        if alloc.kind == "ExternalInput":
            if name != partition_name:
                in_names.append(name)
        elif alloc.kind == "ExternalOutput":
            out_names.append(name)
            shape = tuple(alloc.tensor_shape)
            dtype = mybir.dt.np(alloc.dtype)
            out_avals.append(jax.core.ShapedArray(shape, dtype))
            zero_outs.append(np.zeros(shape, dtype))
    n_params = len(in_names)
    n_outs = len(out_avals)
    all_in_names = list(in_names) + list(out_names)
    if partition_name is not None:
        all_in_names.append(partition_name)

    def _body(*args):
        operands = list(args)
        if partition_name is not None:
            operands.append(partition_id_tensor())
        outs = _bass_exec_p.bind(
            *operands,
            out_avals=tuple(out_avals),
            in_names=tuple(all_in_names),
            out_names=tuple(out_names),
            lowering_input_output_aliases=(),
            sim_require_finite=True,
            sim_require_nnan=True,
            nc=nc,
        )
        return tuple(outs)

    devices = jax.devices()[:n_cores]
    mesh = Mesh(np.asarray(devices), ("core",))
    in_specs = (PartitionSpec("core"),) * (n_params + n_outs)
    out_specs = (PartitionSpec("core"),) * n_outs
    jit_kwargs = {"keep_unused": True}
    if donate:
        jit_kwargs["donate_argnums"] = tuple(range(n_params, n_params + n_outs))
    fn = jax.jit(
        shard_map(_body, mesh=mesh, in_specs=in_specs, out_specs=out_specs,
                  check_rep=False),
        **jit_kwargs,
    )
    sharding = NamedSharding(mesh, PartitionSpec("core"))

    class Runner:
        def __init__(self):
            self.fn = fn
            self.in_names = in_names
            self.out_names = out_names
            self.n_cores = n_cores
            self.sharding = sharding
            self.zero_outs = zero_outs

        def put_inputs(self, in_maps):
            args = []
            for name in in_names:
                cat = np.concatenate([np.asarray(m[name]) for m in in_maps], axis=0)
                args.append(jax.device_put(cat, sharding))
            for z in zero_outs:
                cat = np.concatenate([z] * n_cores, axis=0)
                args.append(jax.device_put(cat, sharding))
            return args

        def __call__(self, args):
            return self.fn(*args)

        def run(self, in_maps):
            args = self.put_inputs(in_maps)
            outs = self.fn(*args)
            jax.block_until_ready(outs)
            res = []
            for c in range(n_cores):
                d = {}
                for i, name in enumerate(out_names):
                    arr = np.asarray(outs[i])
                    per = arr.shape[0] // n_cores
                    d[name] = arr[c * per:(c + 1) * per]
                res.append(d)
            return res

    return Runner()


_CACHE = {}


def kernel(src, dst, vals, x, W1, W2):
    src = np.asarray(src)
    dst = np.asarray(dst)
    vals = np.asarray(vals, dtype=np.float32)
    x = np.asarray(x, dtype=np.float32)
    W1 = np.asarray(W1, dtype=np.float32)
    W2 = np.asarray(W2, dtype=np.float32)

    L = build_layout(src.astype(np.int64), dst.astype(np.int64), vals, NUM_NODES)
    key = (L["NCHUNK1"], L["NCHUNK3"], tuple(L["slabs1"]),
           tuple(L["slabs3"]), tuple(L["first1"]),
           tuple(map(tuple, L["first3"])))
    if key not in _CACHE:
        nc = build_nc(L)
        _CACHE[key] = make_runner(nc)
        _CACHE["r"] = _CACHE[key]
    r = _CACHE[key]
    in_maps = pack_inputs(L, x, W1, W2)
    results = r.run(in_maps)
    return unpack_output(L, results).astype(np.float32)


# revision 21
# speedup vs baseline: 1.2633x; 1.1697x over previous
"""Self-contained Trainium2 Bass kernel for the 2-layer GCN problem.

kernel(src, dst, vals, x, W1, W2) -> [80000, 40] float32 logits,
computed as  A @ (relu((A @ x) @ W1) @ W2)  on 8 NeuronCores.

v3 strategy:
- Nodes sharded round-robin across cores in 128-node slots (by src);
  W1/W2 replicated.
- Phase 1 (z1 = A@x): the per-edge gather of x[dst] is a pure input
  permutation, so the host pre-packs x rows into chunk-slot order and
  the device STREAMS them sequentially (no per-edge DMA descriptors).
  The selection matrices (one-hot(srel)*val per chunk) are likewise
  host-prebuilt and streamed (keeps DVE off the critical path).  A
  matmul per chunk scatter-adds slots into block rows in PSUM, then a
  dense chain per block: z2 = relu(z1 W1) W2 (64 padded cols).
- z2 shards AllGather into a superblock-major z2 table in 3 bank-sized
  pieces so phase-3 gathers of bank b start as soon as piece b lands
  (overlapping later phase-1/2 work).
- Phase 3 (out = A@z2): per-edge dma_gather of 256B z2 rows (4 SWDGE
  queues) + TRANSPOSED accumulation: matmul(lhsT=g[slot,feat],
  rhs=sv[slot,row]) -> accT[feat,row] per superblock in PSUM; bank
  partials summed into an SBUF accumulator.  Output leaves the device
  feature-major; the host transposes while assembling (free).
"""
import numpy as np
import ml_dtypes
import jax
from jax.sharding import Mesh, PartitionSpec, NamedSharding
from jax.experimental.shard_map import shard_map

import concourse.bass as bass
import concourse.bacc as bacc
import concourse.tile as tile
import concourse.mybir as mybir
from concourse import bass2jax
from concourse.bass2jax import _bass_exec_p, install_neuronx_cc_hook, partition_id_tensor
from concourse.masks import make_identity

NUM_NODES = 80000
NUM_EDGES = 1280000

NC, P, GRP = 8, 128, 1024
NBLK = 79            # ceil(80000 / 1024) blocks of 128 rows per core
SUPER = 8
NSB = 10             # ceil(NBLK / SUPER)
SLAB = 48

F_IN = 64
F_HID = 128
F_OUT = 40
FE = 128             # bf16 elements per 256B z2-table row

# z2 table bank structure (block-major): bank b covers blocks
# [BBLO[b], BBHI[b]); table row offset BBASE[b], per-core rows BROWS[b].
# The SMALL bank comes first so AG piece 0 (and with it the phase-3
# gather backbone) starts after only 15/79 of phase 1.
BBLO = [0, 15, 47]
BBHI = [15, 47, 79]
BBASE = [0, 15360, 48128]
BROWS = [1920, 4096, 4096]
NBANK = 3
TABROWS = 80896

bf16 = mybir.dt.bfloat16
f32 = mybir.dt.float32


def _cell_ranks(key, order_key2, nkey):
    """Rank of each edge within its integer cell `key` (cell-internal order
    by order_key2), plus per-cell counts."""
    ord_e = np.lexsort((order_key2, key))
    ks = key[ord_e]
    E = key.shape[0]
    first = np.r_[0, np.flatnonzero(np.diff(ks)) + 1]
    group_start = np.zeros(E, np.int64)
    group_start[first] = first
    group_start = np.maximum.accumulate(group_start)
    rank_sorted = np.arange(E) - group_start
    rank = np.empty(E, np.int64)
    rank[ord_e] = rank_sorted
    cnt = np.bincount(key, minlength=nkey)
    return rank, cnt


def build_layout(src, dst, vals, n_nodes=NUM_NODES):
    src = np.asarray(src, np.int64)
    dst = np.asarray(dst, np.int64)
    vals = np.asarray(vals, np.float32)

    n = np.arange(n_nodes)
    c_of = (n // P) % NC
    j_of = n // GRP
    s_of = n % P

    bank_arr = np.array(BBASE)
    jb = j_of
    bank_of = (jb >= BBLO[1]).astype(np.int64) + (jb >= BBLO[2]).astype(np.int64)
    bblo = np.array(BBLO)[bank_of]
    brows = np.array(BROWS)[bank_of]
    table_row = bank_arr[bank_of] + c_of * brows + (jb - bblo) * P + s_of

    ec, ej, es = c_of[src], j_of[src], s_of[src]
    ebank = bank_of[dst]
    lidx = (table_row[dst] - bank_arr[ebank]).astype(np.int64)

    # ---- phase 1 chunks: cell = (core, block) ----
    key1 = ec * NBLK + ej
    rank1, cnt1 = _cell_ranks(key1, dst, NC * NBLK)
    cnt1 = cnt1.reshape(NC, NBLK)
    K1 = np.maximum((-(-cnt1 // P)).max(axis=0), 1)        # [NBLK]
    CB1 = np.r_[0, np.cumsum(K1)]
    NCHUNK1 = int(CB1[-1])
    chunk_block1 = np.repeat(np.arange(NBLK), K1)
    first1 = CB1[:-1].copy()
    last1 = CB1[1:] - 1
    slot1 = (CB1[ej] + rank1 // P) * P + (rank1 % P)
    slabs1 = []
    for S in range(NSB):
        lo, hi = int(CB1[S * SUPER]), int(CB1[min((S + 1) * SUPER, NBLK)])
        o = lo
        while o < hi:
            take = min(SLAB, hi - o)
            slabs1.append((o, take, S))
            o += take

    # ---- phase 3 chunks: cell = (core, block, bank) ----
    key3 = (ec * NBLK + ej) * NBANK + ebank
    rank3, cnt3 = _cell_ranks(key3, lidx, NC * NBLK * NBANK)
    cnt3 = cnt3.reshape(NC, NBLK, NBANK)
    K3 = np.maximum((-(-cnt3 // P)).max(axis=0), 1)        # [NBLK, NBANK]
    CB3 = np.zeros((NBLK, NBANK), np.int64)
    chunk_block3 = []
    slabs3 = []
    pos = 0
    for b in range(NBANK):
        for S in range(NSB):
            js = range(S * SUPER, min((S + 1) * SUPER, NBLK))
            run_start = pos
            for j in js:
                CB3[j, b] = pos
                pos += int(K3[j, b])
                chunk_block3.extend([j] * int(K3[j, b]))
            o = run_start
            while o < pos:
                take = min(SLAB, pos - o)
                slabs3.append((o, take, b, S))
                o += take
    NCHUNK3 = int(pos)
    chunk_block3 = np.asarray(chunk_block3)
    first3 = CB3
    last3 = CB3 + K3 - 1
    slot3 = (CB3[ej, ebank] + rank3 // P) * P + (rank3 % P)

    # ---- per-core streams ----
    xg_dst = np.zeros((NC, NCHUNK1 * P), np.int64)
    v1 = np.zeros((NC, NCHUNK1 * P), np.float32)
    s1 = np.zeros((NC, NCHUNK1 * P), np.int64)
    # Pad slots must NOT all point at one row: same-address gathers
    # serialize in the DMA engines.  Spread them across the smallest
    # bank's row range (valid for every bank).
    pad_spread = (np.arange(NCHUNK3 * P, dtype=np.int64) * 97) % (NC * min(BROWS))
    gidx3 = np.tile(pad_spread.astype(np.int16), (NC, 1))
    v3 = np.zeros((NC, NCHUNK3 * P), np.float32)
    s3 = np.zeros((NC, NCHUNK3 * P), np.int64)
    for c in range(NC):
        m = ec == c
        sl = slot1[m]
        xg_dst[c, sl] = dst[m]
        v1[c, sl] = vals[m]
        s1[c, sl] = es[m]
        sl3 = slot3[m]
        gidx3[c, sl3] = lidx[m].astype(np.int16)
        v3[c, sl3] = vals[m]
        s3[c, sl3] = es[m]

    return dict(
        NCHUNK1=NCHUNK1, NCHUNK3=NCHUNK3,
        chunk_block1=chunk_block1, chunk_block3=chunk_block3,
        first1=first1, last1=last1, first3=first3, last3=last3,
        slabs1=slabs1, slabs3=slabs3,
        xg_dst=xg_dst, v1=v1, s1=s1,
        gidx3=gidx3, v3=v3, s3=s3,
        c_of=c_of, j_of=j_of, s_of=s_of,
    )


def build_nc(L):
    NCHUNK1, NCHUNK3 = L["NCHUNK1"], L["NCHUNK3"]
    slabs1, slabs3 = L["slabs1"], L["slabs3"]
    cb1, cb3 = L["chunk_block1"], L["chunk_block3"]
    first1, last1 = L["first1"], L["last1"]
    first3, last3 = L["first3"], L["last3"]

    nc = bacc.Bacc("TRN2", target_bir_lowering=False, debug=False, num_devices=NC,
                   num_swdge_queues=4)
    xg = nc.dram_tensor("xg", [NCHUNK1 * P, F_IN], bf16, kind="ExternalInput")
    svs1 = nc.dram_tensor("svs1", [NCHUNK1 * P, P], bf16, kind="ExternalInput")
    gidx3 = nc.dram_tensor("gidx3", [P, NCHUNK3 * 8], mybir.dt.int16, kind="ExternalInput")
    svs3 = nc.dram_tensor("svs3", [NCHUNK3 * P, P], bf16, kind="ExternalInput")
    w1 = nc.dram_tensor("w1", [F_IN, F_HID], bf16, kind="ExternalInput")
    w2 = nc.dram_tensor("w2", [F_HID, 64], bf16, kind="ExternalInput")
    out_ext = nc.dram_tensor("out", [F_IN, NBLK * P], f32, kind="ExternalOutput")

    with tile.TileContext(nc) as tc:
        with (
            tc.tile_pool(name="cons", bufs=1) as cons,
            tc.tile_pool(name="xgp", bufs=3) as xgp,
            tc.tile_pool(name="svt", bufs=4) as svtp,
            tc.tile_pool(name="g3p", bufs=6) as g3p,
            tc.tile_pool(name="dense", bufs=2) as dns,
            tc.tile_pool(name="osb", bufs=1) as osb,
            tc.tile_pool(name="psa", bufs=2, space="PSUM") as psa,
            tc.tile_pool(name="psb", bufs=1, space="PSUM") as psb,
            tc.tile_pool(name="psd", bufs=2, space="PSUM") as psd,
            tc.tile_pool(name="dram", bufs=1, space="DRAM") as dram,
        ):
            ident_t = cons.tile([P, P], bf16)
            make_identity(nc, ident_t[:])
            w1_t = cons.tile([F_IN, F_HID], bf16)
            w2_t = cons.tile([F_HID, 64], bf16)
            idx3_t = cons.tile([P, NCHUNK3 * 8], mybir.dt.int16)
            nc.sync.dma_start(out=w1_t[:], in_=w1[:, :])
            nc.sync.dma_start(out=w2_t[:], in_=w2[:, :])
            nc.sync.dma_start(out=idx3_t[:], in_=gidx3[:, :])

            z2loc = dram.tile([NBLK * P, FE], bf16)
            z2tab = dram.tile([TABROWS, FE], bf16)

            # ---- phase 1: z1 = A@x (streamed), dense chain, z2 shard ----
            def dense_block(j, acc_ap):
                z1_sb = dns.tile([P, F_IN], bf16, tag="z1")
                nc.vector.tensor_copy(out=z1_sb[:], in_=acc_ap)
                pt = psd.tile([F_IN, P], bf16, tag="pt")
                nc.tensor.transpose(out=pt[:], in_=z1_sb[:], identity=ident_t[:])
                z1t = dns.tile([F_IN, P], bf16, tag="z1t")
                nc.vector.tensor_copy(out=z1t[:], in_=pt[:])
                ph = psd.tile([F_HID, P], f32, tag="pd")
                nc.tensor.matmul(out=ph[:], lhsT=w1_t[:], rhs=z1t[:],
                                 start=True, stop=True)
                ht = dns.tile([F_HID, P], bf16, tag="ht")
                nc.vector.tensor_scalar_max(out=ht[:], in0=ph[:], scalar1=0.0)
                pz = psd.tile([P, 64], f32, tag="pd")
                nc.tensor.matmul(out=pz[:], lhsT=ht[:], rhs=w2_t[:],
                                 start=True, stop=True)
                z2_sb = dns.tile([P, 64], bf16, tag="z2")
                nc.vector.tensor_copy(out=z2_sb[:], in_=pz[:])
                nc.sync.dma_start(out=z2loc[j * P:(j + 1) * P, 0:64], in_=z2_sb[:])

            for S in range(NSB):
                jlo = S * SUPER
                jhi = min((S + 1) * SUPER, NBLK)
                acc = psa.tile([P, 64 * SUPER], f32, tag="acc")
                for (c0, Ln, SS) in slabs1:
                    if SS != S:
                        continue
                    g1 = xgp.tile([P, SLAB, F_IN], bf16, tag="xg")
                    nc.sync.dma_start(
                        out=g1[:, 0:Ln, :],
                        in_=xg[c0 * P:(c0 + Ln) * P, :].rearrange(
                            "(p a) f -> p a f", p=P),
                    )
                    svt = svtp.tile([P, SLAB, P], bf16, tag="svt")
                    sv1_eng = nc.sync if (c0 // SLAB) % 3 == 2 else nc.scalar
                    sv1_eng.dma_start(
                        out=svt[:, 0:Ln, :],
                        in_=svs1[c0 * P:(c0 + Ln) * P, :].rearrange(
                            "(p a) f -> p a f", p=P),
                    )
                    for t in range(Ln):
                        ch = c0 + t
                        j = int(cb1[ch])
                        jj = j - jlo
                        nc.tensor.matmul(
                            out=acc[:, 64 * jj:64 * (jj + 1)],
                            lhsT=svt[:, t, :],
                            rhs=g1[:, t, :],
                            start=(ch == first1[j]),
                            stop=(ch == last1[j]),
                            skip_group_check=True,
                        )
                for j in range(jlo, jhi):
                    dense_block(j, acc[:, 64 * (j - jlo):64 * (j - jlo + 1)])

            # ---- phases 2+3 interleaved per bank ----
            out_sb = osb.tile([F_IN, NBLK * P], f32)
            for b in range(NBANK):
                nc.gpsimd.collective_compute(
                    "AllGather",
                    mybir.AluOpType.bypass,
                    replica_groups=[list(range(NC))],
                    ins=[z2loc[BBLO[b] * P:BBHI[b] * P, :].opt()],
                    outs=[z2tab[BBASE[b]:BBASE[b] + NC * BROWS[b], :].opt()],
                )
                for S in range(NSB):
                    jlo = S * SUPER
                    jhi = min((S + 1) * SUPER, NBLK)
                    accT = psb.tile([F_IN, P * SUPER], f32, tag="accT")
                    for si, (c0, Ln, bb, SS) in enumerate(slabs3):
                        if bb != b or SS != S:
                            continue
                        g3 = g3p.tile([P, SLAB, FE], bf16, tag="g3")
                        nc.gpsimd.dma_gather(
                            g3[:, 0:Ln, :],
                            z2tab[BBASE[b]:BBASE[b] + NC * BROWS[b], :],
                            idx3_t[:, c0 * 8:(c0 + Ln) * 8],
                            Ln * P, Ln * P, FE,
                            single_packet=False,
                            queue_num=(si % 4),
                        )
                        svt = svtp.tile([P, SLAB, P], bf16, tag="svt")
                        sv_eng = nc.scalar if (si % 2 == 0) else nc.sync
                        sv_eng.dma_start(
                            out=svt[:, 0:Ln, :],
                            in_=svs3[c0 * P:(c0 + Ln) * P, :].rearrange(
                                "(p a) f -> p a f", p=P),
                        )
                        for t in range(Ln):
                            ch = c0 + t
                            j = int(cb3[ch])
                            jj = j - jlo
                            nc.tensor.matmul(
                                out=accT[:, P * jj:P * (jj + 1)],
                                lhsT=g3[:, t, 0:F_IN],
                                rhs=svt[:, t, :],
                                start=(ch == first3[j, b]),
                                stop=(ch == last3[j, b]),
                                skip_group_check=True,
                            )
                    ncols = P * (jhi - jlo)
                    dst_ap = out_sb[:, P * jlo:P * jhi]
                    if b == 0:
                        nc.vector.tensor_copy(out=dst_ap, in_=accT[:, 0:ncols])
                    elif b == NBANK - 1:
                        nc.vector.tensor_add(out=dst_ap, in0=dst_ap,
                                             in1=accT[:, 0:ncols])
                        nc.sync.dma_start(out=out_ext[:, P * jlo:P * jhi],
                                          in_=dst_ap)
                    else:
                        nc.vector.tensor_add(out=dst_ap, in0=dst_ap,
                                             in1=accT[:, 0:ncols])

    nc.compile()
    return nc


def _slabify(rows, slabs, fdim):
    """Reorder stream rows so each slab is partition-major contiguous:
    dram row slab_base*P + s*Ln + a  <-  slot (c0+a)*P + s."""
    out = np.empty_like(rows)
    for sl in slabs:
        c0, Ln = sl[0], sl[1]
        blk = rows[c0 * P:(c0 + Ln) * P].reshape(Ln, P, fdim)
        out[c0 * P:(c0 + Ln) * P] = blk.transpose(1, 0, 2).reshape(Ln * P, fdim)
    return out


def _sv_rows(s, v, nslots):
    rows = np.zeros((nslots, P), np.float32)
    rows[np.arange(nslots), s] = v
    return rows.astype(ml_dtypes.bfloat16)


def pack_inputs(L, x, W1, W2):
    NCHUNK1, NCHUNK3 = L["NCHUNK1"], L["NCHUNK3"]
    x = np.asarray(x, np.float32)
    w1b = np.asarray(W1, np.float32).astype(ml_dtypes.bfloat16)
    w2b = np.zeros((F_HID, 64), ml_dtypes.bfloat16)
    w2b[:, 0:F_OUT] = np.asarray(W2, np.float32).astype(ml_dtypes.bfloat16)

    xbf = x.astype(ml_dtypes.bfloat16)
    in_maps = []
    for c in range(NC):
        xg_rows = _slabify(xbf[L["xg_dst"][c]], L["slabs1"], F_IN)
        sv1_rows = _slabify(_sv_rows(L["s1"][c], L["v1"][c], NCHUNK1 * P),
                            L["slabs1"], P)
        sv3_rows = _slabify(_sv_rows(L["s3"][c], L["v3"][c], NCHUNK3 * P),
                            L["slabs3"], P)
        n3 = NCHUNK3 * P
        wrap = L["gidx3"][c].reshape(n3 // 16, 16).T      # [16, n/16]
        idx_tile = np.tile(wrap, (8, 1)).copy()
        in_maps.append({
            "xg": xg_rows, "svs1": sv1_rows,
            "gidx3": idx_tile, "svs3": sv3_rows,
            "w1": w1b, "w2": w2b,
        })
    return in_maps


def unpack_output(L, results):
    """results: per-core dicts with 'out' [64, NBLK*P] (feature-major)."""
    big = np.stack([r["out"] for r in results])           # [NC, 64, NBLK*P]
    col = L["j_of"] * P + L["s_of"]
    out = big[L["c_of"], :, col]                          # [NUM_NODES, 64]
    return np.ascontiguousarray(out[:, 0:F_OUT])


def make_runner(nc, n_cores=8, donate=False):
    install_neuronx_cc_hook()
    partition_name = nc.partition_id_tensor.name if nc.partition_id_tensor else None

    in_names, out_names, out_avals, zero_outs = [], [], [], []
    for alloc in nc.m.functions[0].allocations:
        if not isinstance(alloc, mybir.MemoryLocationSet):
            continue
        name = alloc.memorylocations[0].name
        if alloc.kind == "ExternalInput":
            if name != partition_name:
                in_names.append(name)
        elif alloc.kind == "ExternalOutput":
            out_names.append(name)
            shape = tuple(alloc.tensor_shape)
            dtype = mybir.dt.np(alloc.dtype)
            out_avals.append(jax.core.ShapedArray(shape, dtype))
            zero_outs.append(np.zeros(shape, dtype))
    n_params = len(in_names)
    n_outs = len(out_avals)
    all_in_names = list(in_names) + list(out_names)
    if partition_name is not None:
        all_in_names.append(partition_name)

    def _body(*args):
        operands = list(args)
        if partition_name is not None:
            operands.append(partition_id_tensor())
        outs = _bass_exec_p.bind(
            *operands,
            out_avals=tuple(out_avals),
            in_names=tuple(all_in_names),
            out_names=tuple(out_names),
            lowering_input_output_aliases=(),
            sim_require_finite=True,
            sim_require_nnan=True,
            nc=nc,
        )
        return tuple(outs)

    devices = jax.devices()[:n_cores]
    mesh = Mesh(np.asarray(devices), ("core",))
    in_specs = (PartitionSpec("core"),) * (n_params + n_outs)
    out_specs = (PartitionSpec("core"),) * n_outs
    jit_kwargs = {"keep_unused": True}
    if donate:
        jit_kwargs["donate_argnums"] = tuple(range(n_params, n_params + n_outs))
    fn = jax.jit(
        shard_map(_body, mesh=mesh, in_specs=in_specs, out_specs=out_specs,
                  check_rep=False),
        **jit_kwargs,
    )
    sharding = NamedSharding(mesh, PartitionSpec("core"))

    class Runner:
        def __init__(self):
            self.fn = fn
            self.in_names = in_names
            self.out_names = out_names
            self.n_cores = n_cores
            self.sharding = sharding
            self.zero_outs = zero_outs

        def put_inputs(self, in_maps):
            args = []
            for name in in_names:
                cat = np.concatenate([np.asarray(m[name]) for m in in_maps], axis=0)
                args.append(jax.device_put(cat, sharding))
            for z in zero_outs:
                cat = np.concatenate([z] * n_cores, axis=0)
                args.append(jax.device_put(cat, sharding))
            return args

        def __call__(self, args):
            return self.fn(*args)

        def run(self, in_maps):
            args = self.put_inputs(in_maps)
            outs = self.fn(*args)
            jax.block_until_ready(outs)
            res = []
            for c in range(n_cores):
                d = {}
                for i, name in enumerate(out_names):
                    arr = np.asarray(outs[i])
                    per = arr.shape[0] // n_cores
                    d[name] = arr[c * per:(c + 1) * per]
                res.append(d)
            return res

    return Runner()


_CACHE = {}


def kernel(src, dst, vals, x, W1, W2):
    src = np.asarray(src)
    dst = np.asarray(dst)
    vals = np.asarray(vals, dtype=np.float32)
    x = np.asarray(x, dtype=np.float32)
    W1 = np.asarray(W1, dtype=np.float32)
    W2 = np.asarray(W2, dtype=np.float32)

    L = build_layout(src.astype(np.int64), dst.astype(np.int64), vals, NUM_NODES)
    key = (L["NCHUNK1"], L["NCHUNK3"], tuple(L["slabs1"]),
           tuple(L["slabs3"]), tuple(L["first1"]),
           tuple(map(tuple, L["first3"])))
    if key not in _CACHE:
        nc = build_nc(L)
        _CACHE[key] = make_runner(nc)
        _CACHE["r"] = _CACHE[key]
    r = _CACHE[key]
    in_maps = pack_inputs(L, x, W1, W2)
    results = r.run(in_maps)
    return unpack_output(L, results).astype(np.float32)


# revision 23
# speedup vs baseline: 3.9956x; 3.1629x over previous
"""Self-contained Trainium2 Bass kernel for the 2-layer GCN problem.

kernel(src, dst, vals, x, W1, W2) -> [80000, 40] float32 logits,
computed as  A @ (relu((A @ x) @ W1) @ W2)  on 8 NeuronCores.

v3 strategy:
- Nodes sharded round-robin across cores in 128-node slots (by src);
  W1/W2 replicated.
- Phase 1 (z1 = A@x): the per-edge gather of x[dst] is a pure input
  permutation, so the host pre-packs x rows into chunk-slot order and
  the device STREAMS them sequentially (no per-edge DMA descriptors).
  The selection matrices (one-hot(srel)*val per chunk) are likewise
  host-prebuilt and streamed (keeps DVE off the critical path).  A
  matmul per chunk scatter-adds slots into block rows in PSUM, then a
  dense chain per block: z2 = relu(z1 W1) W2 (64 padded cols).
- z2 shards AllGather into a superblock-major z2 table in 3 bank-sized
  pieces so phase-3 gathers of bank b start as soon as piece b lands
  (overlapping later phase-1/2 work).
- Phase 3 (out = A@z2): per-edge dma_gather of 256B z2 rows (4 SWDGE
  queues) + TRANSPOSED accumulation: matmul(lhsT=g[slot,feat],
  rhs=sv[slot,row]) -> accT[feat,row] per superblock in PSUM; bank
  partials summed into an SBUF accumulator.  Output leaves the device
  feature-major; the host transposes while assembling (free).
"""
import numpy as np
import ml_dtypes
import jax
from jax.sharding import Mesh, PartitionSpec, NamedSharding
from jax.experimental.shard_map import shard_map

import concourse.bass as bass
import concourse.bacc as bacc
import concourse.tile as tile
import concourse.mybir as mybir
from concourse import bass2jax
from concourse.bass2jax import _bass_exec_p, install_neuronx_cc_hook, partition_id_tensor
from concourse.masks import make_identity

NUM_NODES = 80000
NUM_EDGES = 1280000

NC, P, GRP = 8, 128, 1024
NBLK = 79            # ceil(80000 / 1024) blocks of 128 rows per core
SUPER = 8
NSB = 10             # ceil(NBLK / SUPER)
SLAB = 48

F_IN = 64
F_HID = 128
F_OUT = 40
FE = 128             # bf16 elements per 256B z2-table row

# z2 table bank structure (block-major): bank b covers blocks
# [BBLO[b], BBHI[b]); table row offset BBASE[b], per-core rows BROWS[b].
# The first bank is a single superblock so AG piece 0 (and with it the
# phase-3 gather backbone) starts after only 8/79 of phase 1.
BBLO = [0, 8, 40, 72]
BBHI = [8, 40, 72, 79]
BBASE = [0, 8192, 40960, 73728]
BROWS = [1024, 4096, 4096, 896]
NBANK = 4
TABROWS = 80896

bf16 = mybir.dt.bfloat16
f32 = mybir.dt.float32


def _cell_ranks(key, order_key2, nkey):
    """Rank of each edge within its integer cell `key` (cell-internal order
    by order_key2), plus per-cell counts."""
    ord_e = np.lexsort((order_key2, key))
    ks = key[ord_e]
    E = key.shape[0]
    first = np.r_[0, np.flatnonzero(np.diff(ks)) + 1]
    group_start = np.zeros(E, np.int64)
    group_start[first] = first
    group_start = np.maximum.accumulate(group_start)
    rank_sorted = np.arange(E) - group_start
    rank = np.empty(E, np.int64)
    rank[ord_e] = rank_sorted
    cnt = np.bincount(key, minlength=nkey)
    return rank, cnt


def build_layout(src, dst, vals, n_nodes=NUM_NODES):
    src = np.asarray(src, np.int64)
    dst = np.asarray(dst, np.int64)
    vals = np.asarray(vals, np.float32)

    n = np.arange(n_nodes)
    c_of = (n // P) % NC
    j_of = n // GRP
    s_of = n % P

    bank_arr = np.array(BBASE)
    jb = j_of
    bank_of = np.digitize(jb, BBLO[1:]).astype(np.int64)
    bblo = np.array(BBLO)[bank_of]
    brows = np.array(BROWS)[bank_of]
    table_row = bank_arr[bank_of] + c_of * brows + (jb - bblo) * P + s_of

    ec, ej, es = c_of[src], j_of[src], s_of[src]
    ebank = bank_of[dst]
    lidx = (table_row[dst] - bank_arr[ebank]).astype(np.int64)

    # ---- phase 1 chunks: cell = (core, block) ----
    key1 = ec * NBLK + ej
    rank1, cnt1 = _cell_ranks(key1, dst, NC * NBLK)
    cnt1 = cnt1.reshape(NC, NBLK)
    K1 = np.maximum((-(-cnt1 // P)).max(axis=0), 1)        # [NBLK]
    CB1 = np.r_[0, np.cumsum(K1)]
    NCHUNK1 = int(CB1[-1])
    chunk_block1 = np.repeat(np.arange(NBLK), K1)
    first1 = CB1[:-1].copy()
    last1 = CB1[1:] - 1
    slot1 = (CB1[ej] + rank1 // P) * P + (rank1 % P)
    slabs1 = []
    for S in range(NSB):
        lo, hi = int(CB1[S * SUPER]), int(CB1[min((S + 1) * SUPER, NBLK)])
        o = lo
        while o < hi:
            take = min(SLAB, hi - o)
            slabs1.append((o, take, S))
            o += take

    # ---- phase 3 chunks: cell = (core, block, bank) ----
    key3 = (ec * NBLK + ej) * NBANK + ebank
    rank3, cnt3 = _cell_ranks(key3, lidx, NC * NBLK * NBANK)
    cnt3 = cnt3.reshape(NC, NBLK, NBANK)
    K3 = np.maximum((-(-cnt3 // P)).max(axis=0), 1)        # [NBLK, NBANK]
    CB3 = np.zeros((NBLK, NBANK), np.int64)
    chunk_block3 = []
    slabs3 = []
    pos = 0
    for b in range(NBANK):
        for S in range(NSB):
            js = range(S * SUPER, min((S + 1) * SUPER, NBLK))
            run_start = pos
            for j in js:
                CB3[j, b] = pos
                pos += int(K3[j, b])
                chunk_block3.extend([j] * int(K3[j, b]))
            o = run_start
            while o < pos:
                take = min(SLAB, pos - o)
                slabs3.append((o, take, b, S))
                o += take
    NCHUNK3 = int(pos)
    chunk_block3 = np.asarray(chunk_block3)
    first3 = CB3
    last3 = CB3 + K3 - 1
    slot3 = (CB3[ej, ebank] + rank3 // P) * P + (rank3 % P)

    # ---- per-core streams ----
    xg_dst = np.zeros((NC, NCHUNK1 * P), np.int64)
    v1 = np.zeros((NC, NCHUNK1 * P), np.float32)
    s1 = np.zeros((NC, NCHUNK1 * P), np.int64)
    # Pad slots must NOT all point at one row: same-address gathers
    # serialize in the DMA engines.  Spread them across the smallest
    # bank's row range (valid for every bank).
    pad_spread = (np.arange(NCHUNK3 * P, dtype=np.int64) * 97) % (NC * min(BROWS))
    gidx3 = np.tile(pad_spread.astype(np.int16), (NC, 1))
    v3 = np.zeros((NC, NCHUNK3 * P), np.float32)
    s3 = np.zeros((NC, NCHUNK3 * P), np.int64)
    for c in range(NC):
        m = ec == c
        sl = slot1[m]
        xg_dst[c, sl] = dst[m]
        v1[c, sl] = vals[m]
        s1[c, sl] = es[m]
        sl3 = slot3[m]
        gidx3[c, sl3] = lidx[m].astype(np.int16)
        v3[c, sl3] = vals[m]
        s3[c, sl3] = es[m]

    return dict(
        NCHUNK1=NCHUNK1, NCHUNK3=NCHUNK3,
        chunk_block1=chunk_block1, chunk_block3=chunk_block3,
        first1=first1, last1=last1, first3=first3, last3=last3,
        slabs1=slabs1, slabs3=slabs3,
        xg_dst=xg_dst, v1=v1, s1=s1,
        gidx3=gidx3, v3=v3, s3=s3,
        c_of=c_of, j_of=j_of, s_of=s_of,
    )


def build_nc(L):
    NCHUNK1, NCHUNK3 = L["NCHUNK1"], L["NCHUNK3"]
    slabs1, slabs3 = L["slabs1"], L["slabs3"]
    cb1, cb3 = L["chunk_block1"], L["chunk_block3"]
    first1, last1 = L["first1"], L["last1"]
    first3, last3 = L["first3"], L["last3"]

    nc = bacc.Bacc("TRN2", target_bir_lowering=False, debug=False, num_devices=NC,
                   num_swdge_queues=4)
    xg = nc.dram_tensor("xg", [NCHUNK1 * P, F_IN], bf16, kind="ExternalInput")
    svs1 = nc.dram_tensor("svs1", [NCHUNK1 * P, P], bf16, kind="ExternalInput")
    gidx3 = nc.dram_tensor("gidx3", [P, NCHUNK3 * 8], mybir.dt.int16, kind="ExternalInput")
    svs3 = nc.dram_tensor("svs3", [NCHUNK3 * P, P], bf16, kind="ExternalInput")
    w1 = nc.dram_tensor("w1", [F_IN, F_HID], bf16, kind="ExternalInput")
    w2 = nc.dram_tensor("w2", [F_HID, 64], bf16, kind="ExternalInput")
    out_ext = nc.dram_tensor("out", [F_IN, NBLK * P], f32, kind="ExternalOutput")

    with tile.TileContext(nc) as tc:
        with (
            tc.tile_pool(name="cons", bufs=1) as cons,
            tc.tile_pool(name="xgp", bufs=3) as xgp,
            tc.tile_pool(name="svt", bufs=4) as svtp,
            tc.tile_pool(name="g3p", bufs=6) as g3p,
            tc.tile_pool(name="dense", bufs=2) as dns,
            tc.tile_pool(name="osb", bufs=1) as osb,
            tc.tile_pool(name="psa", bufs=2, space="PSUM") as psa,
            tc.tile_pool(name="psb", bufs=1, space="PSUM") as psb,
            tc.tile_pool(name="psd", bufs=2, space="PSUM") as psd,
            tc.tile_pool(name="dram", bufs=1, space="DRAM") as dram,
        ):
            ident_t = cons.tile([P, P], bf16)
            make_identity(nc, ident_t[:])
            w1_t = cons.tile([F_IN, F_HID], bf16)
            w2_t = cons.tile([F_HID, 64], bf16)
            idx3_t = cons.tile([P, NCHUNK3 * 8], mybir.dt.int16)
            nc.sync.dma_start(out=w1_t[:], in_=w1[:, :])
            nc.sync.dma_start(out=w2_t[:], in_=w2[:, :])
            nc.sync.dma_start(out=idx3_t[:], in_=gidx3[:, :])

            z2loc = dram.tile([NBLK * P, FE], bf16)
            z2tab = dram.tile([TABROWS, FE], bf16)

            # ---- phase 1: z1 = A@x (streamed), dense chain, z2 shard ----
            def dense_block(j, acc_ap):
                z1_sb = dns.tile([P, F_IN], bf16, tag="z1")
                nc.vector.tensor_copy(out=z1_sb[:], in_=acc_ap)
                pt = psd.tile([F_IN, P], bf16, tag="pt")
                nc.tensor.transpose(out=pt[:], in_=z1_sb[:], identity=ident_t[:])
                z1t = dns.tile([F_IN, P], bf16, tag="z1t")
                nc.vector.tensor_copy(out=z1t[:], in_=pt[:])
                ph = psd.tile([F_HID, P], f32, tag="pd")
                nc.tensor.matmul(out=ph[:], lhsT=w1_t[:], rhs=z1t[:],
                                 start=True, stop=True)
                ht = dns.tile([F_HID, P], bf16, tag="ht")
                nc.vector.tensor_scalar_max(out=ht[:], in0=ph[:], scalar1=0.0)
                pz = psd.tile([P, 64], f32, tag="pd")
                nc.tensor.matmul(out=pz[:], lhsT=ht[:], rhs=w2_t[:],
                                 start=True, stop=True)
                z2_sb = dns.tile([P, 64], bf16, tag="z2")
                nc.vector.tensor_copy(out=z2_sb[:], in_=pz[:])
                nc.sync.dma_start(out=z2loc[j * P:(j + 1) * P, 0:64], in_=z2_sb[:])

            for S in range(NSB):
                jlo = S * SUPER
                jhi = min((S + 1) * SUPER, NBLK)
                acc = psa.tile([P, 64 * SUPER], f32, tag="acc")
                for (c0, Ln, SS) in slabs1:
                    if SS != S:
                        continue
                    g1 = xgp.tile([P, SLAB, F_IN], bf16, tag="xg")
                    nc.sync.dma_start(
                        out=g1[:, 0:Ln, :],
                        in_=xg[c0 * P:(c0 + Ln) * P, :].rearrange(
                            "(p a) f -> p a f", p=P),
                    )
                    svt = svtp.tile([P, SLAB, P], bf16, tag="svt")
                    sv1_eng = nc.sync if (c0 // SLAB) % 3 == 2 else nc.scalar
                    sv1_eng.dma_start(
                        out=svt[:, 0:Ln, :],
                        in_=svs1[c0 * P:(c0 + Ln) * P, :].rearrange(
                            "(p a) f -> p a f", p=P),
                    )
                    for t in range(Ln):
                        ch = c0 + t
                        j = int(cb1[ch])
                        jj = j - jlo
                        nc.tensor.matmul(
                            out=acc[:, 64 * jj:64 * (jj + 1)],
                            lhsT=svt[:, t, :],
                            rhs=g1[:, t, :],
                            start=(ch == first1[j]),
                            stop=(ch == last1[j]),
                            skip_group_check=True,
                        )
                for j in range(jlo, jhi):
                    dense_block(j, acc[:, 64 * (j - jlo):64 * (j - jlo + 1)])

            # ---- phases 2+3 interleaved per bank ----
            out_sb = osb.tile([F_IN, NBLK * P], f32)
            for b in range(NBANK):
                nc.gpsimd.collective_compute(
                    "AllGather",
                    mybir.AluOpType.bypass,
                    replica_groups=[list(range(NC))],
                    ins=[z2loc[BBLO[b] * P:BBHI[b] * P, :].opt()],
                    outs=[z2tab[BBASE[b]:BBASE[b] + NC * BROWS[b], :].opt()],
                )
                for S in range(NSB):
                    jlo = S * SUPER
                    jhi = min((S + 1) * SUPER, NBLK)
                    accT = psb.tile([F_IN, P * SUPER], f32, tag="accT")
                    for si, (c0, Ln, bb, SS) in enumerate(slabs3):
                        if bb != b or SS != S:
                            continue
                        g3 = g3p.tile([P, SLAB, FE], bf16, tag="g3")
                        nc.gpsimd.dma_gather(
                            g3[:, 0:Ln, :],
                            z2tab[BBASE[b]:BBASE[b] + NC * BROWS[b], :],
                            idx3_t[:, c0 * 8:(c0 + Ln) * 8],
                            Ln * P, Ln * P, FE,
                            single_packet=False,
                            queue_num=(si % 4),
                        )
                        svt = svtp.tile([P, SLAB, P], bf16, tag="svt")
                        sv_eng = nc.scalar if (si % 2 == 0) else nc.sync
                        sv_eng.dma_start(
                            out=svt[:, 0:Ln, :],
                            in_=svs3[c0 * P:(c0 + Ln) * P, :].rearrange(
                                "(p a) f -> p a f", p=P),
                        )
                        for t in range(Ln):
                            ch = c0 + t
                            j = int(cb3[ch])
                            jj = j - jlo
                            nc.tensor.matmul(
                                out=accT[:, P * jj:P * (jj + 1)],
                                lhsT=g3[:, t, 0:F_IN],
                                rhs=svt[:, t, :],
                                start=(ch == first3[j, b]),
                                stop=(ch == last3[j, b]),
                                skip_group_check=True,
                            )
                    ncols = P * (jhi - jlo)
                    dst_ap = out_sb[:, P * jlo:P * jhi]
                    if b == 0:
                        nc.vector.tensor_copy(out=dst_ap, in_=accT[:, 0:ncols])
                    elif b == NBANK - 1:
                        nc.vector.tensor_add(out=dst_ap, in0=dst_ap,
                                             in1=accT[:, 0:ncols])
                        nc.sync.dma_start(out=out_ext[:, P * jlo:P * jhi],
                                          in_=dst_ap)
                    else:
                        nc.vector.tensor_add(out=dst_ap, in0=dst_ap,
                                             in1=accT[:, 0:ncols])

    nc.compile()
    return nc


def _slabify(rows, slabs, fdim):
    """Reorder stream rows so each slab is partition-major contiguous:
    dram row slab_base*P + s*Ln + a  <-  slot (c0+a)*P + s."""
    out = np.empty_like(rows)
    for sl in slabs:
        c0, Ln = sl[0], sl[1]
        blk = rows[c0 * P:(c0 + Ln) * P].reshape(Ln, P, fdim)
        out[c0 * P:(c0 + Ln) * P] = blk.transpose(1, 0, 2).reshape(Ln * P, fdim)
    return out


def _sv_rows(s, v, nslots):
    rows = np.zeros((nslots, P), np.float32)
    rows[np.arange(nslots), s] = v
    return rows.astype(ml_dtypes.bfloat16)


def pack_inputs(L, x, W1, W2):
    NCHUNK1, NCHUNK3 = L["NCHUNK1"], L["NCHUNK3"]
    x = np.asarray(x, np.float32)
    w1b = np.asarray(W1, np.float32).astype(ml_dtypes.bfloat16)
    w2b = np.zeros((F_HID, 64), ml_dtypes.bfloat16)
    w2b[:, 0:F_OUT] = np.asarray(W2, np.float32).astype(ml_dtypes.bfloat16)

    xbf = x.astype(ml_dtypes.bfloat16)
    in_maps = []
    for c in range(NC):
        xg_rows = _slabify(xbf[L["xg_dst"][c]], L["slabs1"], F_IN)
        sv1_rows = _slabify(_sv_rows(L["s1"][c], L["v1"][c], NCHUNK1 * P),
                            L["slabs1"], P)
        sv3_rows = _slabify(_sv_rows(L["s3"][c], L["v3"][c], NCHUNK3 * P),
                            L["slabs3"], P)
        n3 = NCHUNK3 * P
        wrap = L["gidx3"][c].reshape(n3 // 16, 16).T      # [16, n/16]
        idx_tile = np.tile(wrap, (8, 1)).copy()
        in_maps.append({
            "xg": xg_rows, "svs1": sv1_rows,
            "gidx3": idx_tile, "svs3": sv3_rows,
            "w1": w1b, "w2": w2b,
        })
    return in_maps


def unpack_output(L, results):
    """results: per-core dicts with 'out' [64, NBLK*P] (feature-major)."""
    big = np.stack([r["out"] for r in results])           # [NC, 64, NBLK*P]
    col = L["j_of"] * P + L["s_of"]
    out = big[L["c_of"], :, col]                          # [NUM_NODES, 64]
    return np.ascontiguousarray(out[:, 0:F_OUT])


def make_runner(nc, n_cores=8, donate=False):
    install_neuronx_cc_hook()
    partition_name = nc.partition_id_tensor.name if nc.partition_id_tensor else None

    in_names, out_names, out_avals, zero_outs = [], [], [], []
    for alloc in nc.m.functions[0].allocations:
        if not isinstance(alloc, mybir.MemoryLocationSet):
            continue
        name = alloc.memorylocations[0].name
        if alloc.kind == "ExternalInput":
            if name != partition_name:
                in_names.append(name)
        elif alloc.kind == "ExternalOutput":
            out_names.append(name)
            shape = tuple(alloc.tensor_shape)
            dtype = mybir.dt.np(alloc.dtype)
            out_avals.append(jax.core.ShapedArray(shape, dtype))
            zero_outs.append(np.zeros(shape, dtype))
    n_params = len(in_names)
    n_outs = len(out_avals)
    all_in_names = list(in_names) + list(out_names)
    if partition_name is not None:
        all_in_names.append(partition_name)

    def _body(*args):
        operands = list(args)
        if partition_name is not None:
            operands.append(partition_id_tensor())
        outs = _bass_exec_p.bind(
            *operands,
            out_avals=tuple(out_avals),
            in_names=tuple(all_in_names),
            out_names=tuple(out_names),
            lowering_input_output_aliases=(),
            sim_require_finite=True,
            sim_require_nnan=True,
            nc=nc,
        )
        return tuple(outs)

    devices = jax.devices()[:n_cores]
    mesh = Mesh(np.asarray(devices), ("core",))
    in_specs = (PartitionSpec("core"),) * (n_params + n_outs)
    out_specs = (PartitionSpec("core"),) * n_outs
    jit_kwargs = {"keep_unused": True}
    if donate:
        jit_kwargs["donate_argnums"] = tuple(range(n_params, n_params + n_outs))
    fn = jax.jit(
        shard_map(_body, mesh=mesh, in_specs=in_specs, out_specs=out_specs,
                  check_rep=False),
        **jit_kwargs,
    )
    sharding = NamedSharding(mesh, PartitionSpec("core"))

    class Runner:
        def __init__(self):
            self.fn = fn
            self.in_names = in_names
            self.out_names = out_names
            self.n_cores = n_cores
            self.sharding = sharding
            self.zero_outs = zero_outs

        def put_inputs(self, in_maps):
            args = []
            for name in in_names:
                cat = np.concatenate([np.asarray(m[name]) for m in in_maps], axis=0)
                args.append(jax.device_put(cat, sharding))
            for z in zero_outs:
                cat = np.concatenate([z] * n_cores, axis=0)
                args.append(jax.device_put(cat, sharding))
            return args

        def __call__(self, args):
            return self.fn(*args)

        def run(self, in_maps):
            args = self.put_inputs(in_maps)
            outs = self.fn(*args)
            jax.block_until_ready(outs)
            res = []
            for c in range(n_cores):
                d = {}
                for i, name in enumerate(out_names):
                    arr = np.asarray(outs[i])
                    per = arr.shape[0] // n_cores
                    d[name] = arr[c * per:(c + 1) * per]
                res.append(d)
            return res

    return Runner()


_CACHE = {}


def kernel(src, dst, vals, x, W1, W2):
    src = np.asarray(src)
    dst = np.asarray(dst)
    vals = np.asarray(vals, dtype=np.float32)
    x = np.asarray(x, dtype=np.float32)
    W1 = np.asarray(W1, dtype=np.float32)
    W2 = np.asarray(W2, dtype=np.float32)

    L = build_layout(src.astype(np.int64), dst.astype(np.int64), vals, NUM_NODES)
    key = (L["NCHUNK1"], L["NCHUNK3"], tuple(L["slabs1"]),
           tuple(L["slabs3"]), tuple(L["first1"]),
           tuple(map(tuple, L["first3"])))
    if key not in _CACHE:
        nc = build_nc(L)
        _CACHE[key] = make_runner(nc)
        _CACHE["r"] = _CACHE[key]
    r = _CACHE[key]
    in_maps = pack_inputs(L, x, W1, W2)
    results = r.run(in_maps)
    return unpack_output(L, results).astype(np.float32)
